# revision 27
# baseline (speedup 1.0000x reference)
"""
Causal masked scaled-dot-product attention on 8 Trainium2 NeuronCores.

Problem: B=16, S=2048, D_K=D_V=128, fp32.
  scores = Q @ K^T / sqrt(128); mask j>i with -1e9; softmax; out = P @ V.

Sharding: batch dim B=16 split across 8 cores (2 batches per core).

Per-core kernel design:
  - Host pre-transposes Q,K to [D, S] (f16). Scores computed TRANSPOSED:
    ST[t, s] = K[t,:] . Q[s,:] via lhsT = K-chunk [d, 128], rhs = QT.
  - s in 4 blocks of 512 (one PSUM bank each for O^T accumulation); t in
    chunks of 128, processed in PAIRS per block: two ST matmuls write
    adjacent regions of one [128, <=1024] PSUM tile (2 banks); ONE
    scalar-engine exp covers the pair, halving the +352-cycle
    per-instruction overhead of the activation engine (a bottleneck).
  - Diagonal chunks trimmed (widths 512/384/256/128) and masked
    POST-exp by a 0/1 triangle multiply of E[:, :128] on the vector
    engine (exact zeros, no PSUM mask adds).
  - Rowsum via "broadcast" matmuls on the PE: lhsT = ones [128,128],
    rhs = E chunk, accumulated in PSUM across the block -> rowsum
    replicated on all 128 partitions. reciprocal_approx_fast (DVE,
    ~18-bit) -> tensor_mul normalizes O^T. No transposes, no rank-1
    broadcasts, no elementwise E accumulation streams.
  - ALL matmul weights are [128,128] f16, which lets walrus's
    --enable-ldw-opt=true overlap every LDWEIGHTS with the in-flight
    matmul stream (the flag rejects fp32/odd-shaped weight loads; with
    the uniform mix it compiles). Without it LDWEIGHTS serializes and
    costs ~27us of PE time.
  - PV/rowsum consumption and block finalize lag LAG halves behind
    score/exp production in one GLOBAL pipeline across block and batch
    boundaries, so the PE never drains at a block edge; block order
    ascends so loads stream in consumption order and the kernel ends on
    the biggest block, whose PE work hides earlier finalize chains. The
    very last block's normalize+store runs piecewise so the final DMAs
    overlap the normalize chain.
  - Outputs are stored f16 (error budget 2e-2; this costs ~2e-4) to
    halve store-DMA traffic; the host upcasts. Loads stay on the
    sync/gpsimd queues and stores dispatch from the scalar engine --
    sharing a queue makes batch-1 load dispatches wait behind
    batch-0 store dispatches (which block on normalize results).
  - f16 warm-up matmuls keep the PE's HAM clock gate at full rate
    through the initial DMA window.

Output is produced transposed [D, S] per batch; host transposes back.
"""

import math
import os
import sys

import numpy as np

_REPO = "/opt/trn_rl_repo"
if _REPO not in sys.path:
    sys.path.insert(0, _REPO)

import concourse.bass as bass  # noqa: E402
import concourse.tile as tile  # noqa: E402
from concourse import bacc  # noqa: E402
from concourse import mybir  # noqa: E402
from concourse.bass_utils import run_bass_kernel_spmd  # noqa: E402

F32 = mybir.dt.float32
F16 = mybir.dt.float16
EXP = mybir.ActivationFunctionType.Exp

B, S, D = 16, 2048, 128
N_CORES = 8
BPC = B // N_CORES  # batches per core
NK = S // 512  # s-blocks per batch
NT = S // 128  # t-chunks per batch


def build_attention(nc, tc, ctx):
    scale = 1.0 / math.sqrt(D)

    QT = nc.dram_tensor("QT", [BPC, D, S], F16, kind="ExternalInput").ap()
    KT = nc.dram_tensor("KT", [BPC, D, S], F16, kind="ExternalInput").ap()
    V = nc.dram_tensor("V", [BPC, S, D], F16, kind="ExternalInput").ap()
    MASK01 = nc.dram_tensor("MASK01", [128, 128], F16, kind="ExternalInput").ap()
    ONES = nc.dram_tensor("ONES", [128, 128], F16, kind="ExternalInput").ap()
    OT = nc.dram_tensor("OT", [BPC, D, S], F16, kind="ExternalOutput").ap()

    singles = ctx.enter_context(tc.tile_pool(name="singles", bufs=1))
    qkv_pool = ctx.enter_context(tc.tile_pool(name="qkv", bufs=2))
    epool = ctx.enter_context(tc.tile_pool(name="epool", bufs=7))
    small = ctx.enter_context(tc.tile_pool(name="small", bufs=3))
    opool = ctx.enter_context(tc.tile_pool(name="osb", bufs=4))
    ps_s = ctx.enter_context(tc.tile_pool(name="ps_scores", bufs=2, space="PSUM"))
    ps_o = ctx.enter_context(tc.tile_pool(name="ps_o", bufs=2, space="PSUM"))
    ps_r = ctx.enter_context(tc.tile_pool(name="ps_r", bufs=2, space="PSUM"))

    mask01_sb = singles.tile([128, 128], F16, tag="mask01")
    nc.gpsimd.dma_start(out=mask01_sb, in_=MASK01)
    ones_sb = singles.tile([128, 128], F16, tag="ones")
    nc.gpsimd.dma_start(out=ones_sb, in_=ONES)

    # Warm-up: dummy f16 matmuls on zeroed SBUF while input DMAs are in
    # flight; the PE's HAM clock gate needs ~3.4us of sustained activity
    # to reach full rate. (f16 so every LDWEIGHTS in the program is a
    # [128,128] f16 load -- a requirement of walrus's ldw-opt.)
    warm_sb = singles.tile([128, 512], F16, tag="warm")
    nc.vector.memset(warm_sb, 0.0)

    def warm(n):
        for _ in range(n):
            warm_ps = ps_s.tile([128, 512], F32, tag="sc")
            nc.tensor.matmul(
                warm_ps, lhsT=warm_sb[:, 0:128], rhs=warm_sb, start=True, stop=True
            )

    warm(16)

    PIECES = [(512 * i, 512) for i in range(4)]

    # Global pipeline state: PV/rowsum consumption and block finalize lag
    # behind score/exp production by LAG halves, ACROSS block and batch
    # boundaries, so the PE never drains at a block edge.
    pending = []
    LAG = 4

    def consume(item):
        blk, c, off, w, e_ap = item
        blk["n"] += 1
        first = blk["n"] == 1
        last = blk["n"] == blk["nhalves"]
        nc.tensor.matmul(
            blk["o"][:, off : off + w],
            lhsT=blk["vt"](c),
            rhs=e_ap,
            start=first,
            stop=last,
        )
        nc.tensor.matmul(
            blk["r"][:, off : off + w],
            lhsT=ones_sb,
            rhs=e_ap,
            start=first,
            stop=last,
        )
        if last:
            blk["finalize"]()

    for b in range(BPC):
        # ascending blocks for BOTH batches: loads stream in block order,
        # and the kernel ends on the biggest block, whose PE work hides
        # the preceding blocks' finalize chains
        korder = list(range(NK))

        tiles = {}

        def load(kind, idx, engine):
            lo, w = PIECES[idx]
            t = qkv_pool.tile([128, w], F16, tag=f"{kind}{idx}")
            if kind == "vt":
                engine.dma_start(
                    out=t.rearrange("p (c v) -> p c v", v=128),
                    in_=V[b][lo : lo + w].rearrange("(c p) v -> p c v", p=128),
                )
            else:
                src_ap = (QT if kind == "qt" else KT)[b][:, lo : lo + w]
                engine.dma_start(out=t, in_=src_ap)
            tiles[(kind, idx)] = t

        for i in range(len(PIECES)):
            load("kt", i, nc.sync)
            load("qt", i, nc.gpsimd)
            load("vt", i, nc.sync)

        def kt_chunk(c):
            for i, (lo, w) in enumerate(PIECES):
                if lo <= 128 * c < lo + w:
                    return tiles[("kt", i)][:, 128 * c - lo : 128 * c - lo + 128]
            raise AssertionError

        def vt_chunk(c, tiles=tiles):
            # bind THIS batch's tiles dict: vt_chunk is called lazily from
            # consume(), possibly after the next batch rebinds `tiles`
            for i, (lo, w) in enumerate(PIECES):
                if lo <= 128 * c < lo + w:
                    t = tiles[("vt", i)].rearrange("p (c v) -> p c v", v=128)
                    return t[:, c - lo // 128]
            raise AssertionError

        def qt_block(k, off):
            for i, (lo, w) in enumerate(PIECES):
                if lo <= 512 * k < lo + w:
                    o0 = 512 * k - lo + off
                    return tiles[("qt", i)][:, o0 : o0 + 512 - off]
            raise AssertionError

        for ki, k in enumerate(korder):
            o_ps = ps_o.tile([128, 512], F32, tag="o")
            r_ps = ps_r.tile([128, 512], F32, tag="r")

            # pair list: halves are (chunk, off); diag trims off = 128*m
            pairs = []
            for p in range(2 * k):
                pairs.append(((2 * p, 0), (2 * p + 1, 0), False))
            pairs.append(((4 * k, 0), (4 * k + 1, 128), True))
            pairs.append(((4 * k + 2, 256), (4 * k + 3, 384), True))

            def make_finalize(b, k, ki, o_ps, r_ps):
                def finalize():
                    rb = small.tile([128, 512], F32, tag="rb", name=f"rb{k}")
                    nc.vector.reciprocal_approx_fast(rb, r_ps)
                    out_sb = opool.tile([128, 512], F16, tag="out", name=f"os{k}")
                    if ki == NK - 1 and b == BPC - 1:
                        # exposed tail: normalize + store piecewise so the
                        # final DMAs overlap the normalize chain
                        for cc in range(4):
                            sl = slice(128 * cc, 128 * (cc + 1))
                            nc.vector.tensor_mul(
                                out_sb[:, sl], o_ps[:, sl], rb[:, sl]
                            )
                            nc.scalar.dma_start(
                                out=OT[b][
                                    :,
                                    512 * k + 128 * cc : 512 * k + 128 * (cc + 1),
                                ],
                                in_=out_sb[:, sl],
                            )
                    else:
                        nc.vector.tensor_mul(out_sb, o_ps, rb)
                        nc.scalar.dma_start(
                            out=OT[b][:, 512 * k : 512 * (k + 1)], in_=out_sb
                        )

                return finalize

            blk = {
                "o": o_ps,
                "r": r_ps,
                "vt": vt_chunk,
                "n": 0,
                "nhalves": len(pairs) * 2,
                "finalize": make_finalize(b, k, ki, o_ps, r_ps),
            }

            for h0, h1, diag in pairs:
                widths = [512 - h0[1], 512 - h1[1]]
                ew = widths[0] + widths[1]
                ps = ps_s.tile([128, 1024], F32, tag="sc")
                col = 0
                for (c, off), w in zip((h0, h1), widths):
                    nc.tensor.matmul(
                        ps[:, col : col + w],
                        lhsT=kt_chunk(c),
                        rhs=qt_block(k, off),
                        start=True,
                        stop=True,
                    )
                    col += w
                e = epool.tile([128, 1024], F16, tag="e")
                nc.scalar.activation(e[:, :ew], ps[:, :ew], EXP, scale=scale)
                if diag:
                    nc.vector.tensor_mul(e[:, 0:128], e[:, 0:128], mask01_sb)
                    c0 = widths[0]
                    nc.vector.tensor_mul(
                        e[:, c0 : c0 + 128], e[:, c0 : c0 + 128], mask01_sb
                    )
                col = 0
                for (c, off), w in zip((h0, h1), widths):
                    pending.append((blk, c, off, w, e[:, col : col + w]))
                    col += w
                while len(pending) > LAG:
                    consume(pending.pop(0))
    while pending:
        consume(pending.pop(0))


def _compile_with_ldw_opt(nc):
    """Compile nc with walrus's --enable-ldw-opt=true (overlaps LDWEIGHTS
    with the in-flight matmul stream). The flag is only compatible with
    uniform [128,128] non-fp32 weight loads, which this program has.
    The run_command patch is scoped to this compile and restored after."""
    import concourse.bass_utils as _bu

    orig = _bu.run_command

    def patched(cmd, *a, **kw):
        cmd = [
            "--enable-ldw-opt=true" if c == "--enable-ldw-opt=false" else c
            for c in cmd
        ]
        return orig(cmd, *a, **kw)

    _bu.run_command = patched
    try:
        nc.compile()
    finally:
        _bu.run_command = orig


_CACHE = {}


def _get_nc():
    key = "v4"
    if key not in _CACHE:
        from contextlib import ExitStack

        nc = bacc.Bacc("TRN2", target_bir_lowering=False, debug=False)
        with tile.TileContext(nc) as tc, ExitStack() as ctx:
            build_attention(nc, tc, ctx)
        if os.environ.get("ATTN_NO_LDW_OPT"):
            nc.compile()
        else:
            _compile_with_ldw_opt(nc)
        _CACHE[key] = nc
    return _CACHE[key]


LAST_RESULTS = None  # BassKernelResults of the most recent kernel() call


def _install_ntff_hook():
    """Provide antenv.axon_hooks (absent in this image) so that
    run_bass_kernel_spmd(trace=True) can capture NTFF profiles via the
    axon .so."""
    import types

    import antenv

    if "antenv.axon_hooks" not in sys.modules:
        mod = types.ModuleType("antenv.axon_hooks")
        state = {"hook": None}
        mod.set_axon_ntff_profile_hook = lambda h: state.__setitem__("hook", h)
        mod.get_axon_ntff_profile_hook = lambda: state["hook"]
        sys.modules["antenv.axon_hooks"] = mod
        antenv.axon_hooks = mod
    mod = sys.modules["antenv.axon_hooks"]
    if mod.get_axon_ntff_profile_hook() is None:
        from trn_agent_boot.trn_boot import _ntff_profile_via_ctypes

        mod.set_axon_ntff_profile_hook(
            _ntff_profile_via_ctypes("/opt/axon/libaxon_pjrt.so")
        )


def kernel(Q, K, V):
    global LAST_RESULTS
    Q = np.ascontiguousarray(np.asarray(Q, dtype=np.float32))
    K = np.ascontiguousarray(np.asarray(K, dtype=np.float32))
    V = np.ascontiguousarray(np.asarray(V, dtype=np.float32))
    assert Q.shape == (B, S, D), Q.shape

    nc = _get_nc()

    i = np.arange(128)
    mask01 = (i[None, :] >= i[:, None]).astype(np.float16)
    ones = np.ones((128, 128), dtype=np.float16)

    QTf = Q.transpose(0, 2, 1).astype(np.float16)
    KTf = K.transpose(0, 2, 1).astype(np.float16)
    Vf = V.astype(np.float16)
    in_maps = []
    for c in range(N_CORES):
        sl = slice(BPC * c, BPC * (c + 1))
        in_maps.append(
            {
                "QT": np.ascontiguousarray(QTf[sl]),
                "KT": np.ascontiguousarray(KTf[sl]),
                "V": np.ascontiguousarray(Vf[sl]),
                "MASK01": mask01,
                "ONES": ones,
            }
        )

    trace = bool(int(os.environ.get("ATTN_TRACE", "0")))
    if trace:
        _install_ntff_hook()
    res = run_bass_kernel_spmd(nc, in_maps, list(range(N_CORES)), trace=trace)
    LAST_RESULTS = res

    out = np.empty((B, S, D), dtype=np.float32)
    for c in range(N_CORES):
        for b in range(BPC):
            out[BPC * c + b] = res.results[c]["OT"][b].T.astype(np.float32)
    return out
